# revision 43
# baseline (speedup 1.0000x reference)
"""Trainium2 Bass kernel for nn_BinaryTokenClassificationModel (segment_reduce).

Math: logits[b,i,j] = dot(segmean(1+i), w_src) + dot(segmean(513+j), w_tgt) + bias,
where segmean(s) is the mean of outputs[b] over the s-th consecutive run of equal
word_ids (attention_mask is all ones for this problem).  dot commutes with the
segment mean, so the kernel computes per-token projections proj[t,c]=x[t]·w_c via
PE (block transpose + matmul), then does the ragged segment-sum of the scalar
projections with a factored one-hot matmul (s_lo=seg%128 one-hot as stationary,
s_hi one-hot * proj as moving), and assembles the [512,512] broadcast-add output
with tiny selector matmuls.  Tokens whose segment id exceeds 1024 can never
influence the output, so only the first NT*128 tokens (host-computed cutoff) are
ever loaded — the DMA roofline drops accordingly.

Sharding: pure data parallel, one example (B=8) per NeuronCore (8 cores).
"""
import sys

for _p in ("/opt/trn_rl_repo", "/root/.axon_site/_ro/trn_rl_repo"):
    if _p not in sys.path:
        sys.path.append(_p)

from contextlib import ExitStack

import numpy as np

import concourse.bacc as bacc
import concourse.bass as bass
import concourse.tile as tile
from concourse import mybir
from concourse.bass_utils import run_bass_kernel_spmd

F32 = mybir.dt.float32
BF16 = mybir.dt.bfloat16
P = 128
H = 1024
HC = H // P          # 8 h-chunks
NSH = 9              # s_hi one-hot width (covers segments 0..1151 >= 1..1024 needed)
NR = 3 * NSH         # pooling rhs width: (src, tgt, count) x 9
AL = mybir.AluOpType


def _build_nc(NT: int, modes: list[str]) -> bass.Bass:
    nc = bacc.Bacc("TRN2", target_bir_lowering=False, debug=False, num_devices=8)
    NCC = 5 * P + 2 * NT + 1 + 10
    x_d = nc.declare_dram_parameter("x", [NT * P, H], F32, isOutput=False)
    cc_d = nc.declare_dram_parameter("consts", [P, NCC], F32, isOutput=False)
    wb_d = nc.declare_dram_parameter("wrepb", [P, 2 * H], F32, isOutput=False)
    y_d = nc.declare_dram_parameter("y", [512, 512], F32, isOutput=True)

    with tile.TileContext(nc) as tc, ExitStack() as ctx:
        consts = ctx.enter_context(tc.tile_pool(name="consts", bufs=1))
        segp = ctx.enter_context(tc.tile_pool(name="segp", bufs=1))
        xpool = ctx.enter_context(tc.tile_pool(name="xp", bufs=4))
        scrp = ctx.enter_context(tc.tile_pool(name="scr", bufs=3))
        clpool = ctx.enter_context(tc.tile_pool(name="clp", bufs=3))
        rpool = ctx.enter_context(tc.tile_pool(name="rp", bufs=3))
        vpool = ctx.enter_context(tc.tile_pool(name="vp", bufs=NT))
        opool = ctx.enter_context(tc.tile_pool(name="op", bufs=2))
        ppool_acc = ctx.enter_context(tc.tile_pool(name="pacc", bufs=1, space="PSUM"))
        ppool_sm = ctx.enter_context(tc.tile_pool(name="psm", bufs=4, space="PSUM"))

        # ---- load all constants / small inputs in one DMA (on the scalar
        # HWDGE queue so the x loads on the sync queue start immediately) ----
        cc = consts.tile([P, NCC], F32)
        nc.scalar.dma_start(out=cc, in_=cc_d[:])
        ident = cc[:, 0:P]
        lt = cc[:, P:2 * P]
        s1 = cc[:, 2 * P:3 * P]
        s2 = cc[:, 3 * P:4 * P]
        iota = cc[:, 4 * P:5 * P]
        widf = cc[:, 5 * P:5 * P + NT]
        widp = cc[:, 5 * P + NT:5 * P + 2 * NT]
        thr = cc[:, 5 * P + 2 * NT:5 * P + 2 * NT + 10]
        bias = cc[0:1, NCC - 1:NCC]
        wrep = consts.tile([P, 2 * H], F32)        # [128, 2048]: w_src | w_tgt replicated rows
        nc.scalar.dma_start(out=wrep, in_=wb_d[:])

        # ---- segment ids: seg = cumsum(new_seg) - 1, token t = 128*i + p at [p, i] ----
        new_seg = segp.tile([P, NT], F32)
        nc.vector.tensor_tensor(out=new_seg, in0=widf, in1=widp, op=AL.not_equal)

        tot_ps = ppool_sm.tile([1, NT], F32, tag="sm")
        # column totals: all-ones column is LT[:, 127]
        nc.tensor.matmul(tot_ps, lhsT=lt[:, P - 1:P], rhs=new_seg, start=True, stop=True)
        sc_a = segp.tile([1, NT], F32)
        sc_b = segp.tile([1, NT], F32)
        nc.vector.tensor_copy(out=sc_a, in_=tot_ps)
        # inclusive prefix over the NT columns (log-shift adds)
        k = 1
        cur, nxt = sc_a, sc_b
        while k < NT:
            nc.vector.tensor_tensor(out=nxt[:, k:], in0=cur[:, k:], in1=cur[:, :NT - k], op=AL.add)
            nc.vector.tensor_copy(out=nxt[:, :k], in_=cur[:, :k])
            cur, nxt = nxt, cur
            k *= 2
        segoff = segp.tile([1, NT], F32)
        nc.vector.memset(segoff, -1.0)
        if NT > 1:
            nc.vector.tensor_scalar(out=segoff[:, 1:], in0=cur[:, :NT - 1], scalar1=-1.0, scalar2=None, op0=AL.add)

        cum_ps = ppool_sm.tile([P, NT], F32, tag="sm")
        nc.tensor.matmul(cum_ps, lhsT=lt, rhs=new_seg, start=True, stop=False)
        # broadcast segoff over partitions: ones-row is LT[0:1, :]
        nc.tensor.matmul(cum_ps, lhsT=lt[0:1, :], rhs=segoff, start=False, stop=True)
        seg = segp.tile([P, NT], F32)
        nc.vector.tensor_copy(out=seg, in_=cum_ps)
        # cmp_hi staircase: ge10[p,i,u] = (seg >= 128*(u+1)); shi = sum_u ge10;
        # cmp_hi[u] = ge10[u-1] - ge10[u] (u>=1), 1 - ge10[0] (u=0); slo = seg - 128*shi
        ge10 = segp.tile([P, NT, 10], F32)
        nc.vector.tensor_tensor(
            out=ge10,
            in0=seg.unsqueeze(2).to_broadcast((P, NT, 10)),
            in1=thr.unsqueeze(1).to_broadcast((P, NT, 10)),
            op=AL.is_ge)
        ch_all = segp.tile([P, NT, NSH], F32)
        nc.vector.tensor_scalar(out=ch_all[:, :, 0], in0=ge10[:, :, 0],
                                scalar1=-1.0, scalar2=1.0, op0=AL.mult, op1=AL.add)
        nc.vector.tensor_tensor(out=ch_all[:, :, 1:NSH], in0=ge10[:, :, 0:NSH - 1],
                                in1=ge10[:, :, 1:NSH], op=AL.subtract)
        shi = segp.tile([P, NT], F32)
        nc.vector.tensor_reduce(out=shi, in_=ge10, axis=mybir.AxisListType.X, op=AL.add)
        slo = segp.tile([P, NT], F32)
        nc.vector.tensor_scalar(out=slo, in0=shi, scalar1=-128.0, scalar2=None, op0=AL.mult)
        nc.vector.tensor_tensor(out=slo, in0=slo, in1=seg, op=AL.add)

        # ---- main loop over token tiles ----
        # proj[t, c] = x[t] . w_c via DVE multiply + ACT fused reduce; the
        # src/tgt crossover (host-computed per tile) avoids computing both
        # dots for most tiles.
        pool_ps = ppool_acc.tile([NR, P], F32)
        # pass 1: stream x, multiply by w (DVE), reduce to per-token dots (ACT)
        vts = []
        for g in range(NT // 2):
            x_pair = xpool.tile([P, 2, H], F32)
            src = x_d[256 * g:256 * (g + 1), :].rearrange("(two p) h -> p two h", p=P)
            # alternate the two HWDGE queues for issue parallelism
            (nc.sync if g % 2 == 0 else nc.scalar).dma_start(out=x_pair, in_=src)
            for half in range(2):
                i = 2 * g + half
                x_sub = x_pair[:, half, :]
                v = vpool.tile([P, 2], F32)
                vts.append(v)
                nc.vector.memset(v, 0.0)
                for c in range(2):
                    if (c == 0 and modes[i] == "tgt") or (c == 1 and modes[i] == "src"):
                        continue
                    scr = scrp.tile([P, H], F32)
                    nc.vector.tensor_tensor(out=scr, in0=x_sub, in1=wrep[:, c * H:(c + 1) * H], op=AL.mult)
                    nc.scalar.activation(out=scr, in_=scr, func=mybir.ActivationFunctionType.Copy,
                                         accum_out=v[:, c:c + 1])
        # pass 2: one-hot compares, pooling rhs, accumulate pool^T[(u,c), s_lo]
        for i in range(NT):
            cl = clpool.tile([P, P], F32)
            nc.vector.tensor_scalar(out=cl, in0=iota, scalar1=slo[:, i:i + 1], scalar2=None, op0=AL.is_equal)
            ch = ch_all[:, i, :]
            r_t = rpool.tile([P, NSH, 3], F32, tag="r")
            nc.vector.tensor_tensor(
                out=r_t[:, :, 0:2],
                in0=ch.unsqueeze(2).to_broadcast((P, NSH, 2)),
                in1=vts[i].unsqueeze(1).to_broadcast((P, NSH, 2)),
                op=AL.mult)
            nc.vector.tensor_copy(out=r_t[:, :, 2], in_=ch)
            nc.tensor.matmul(pool_ps, lhsT=r_t.rearrange("p u c -> p (u c)"), rhs=cl,
                             start=(i == 0), stop=(i == NT - 1), skip_group_check=True)

        # ---- tail: means, extraction, broadcast-add ----
        # pool_ps is [(u,c), s_lo]; transpose back to [s_lo, (u,c)] via PE
        poolT_sb = segp.tile([NR, P], F32)
        nc.vector.tensor_copy(out=poolT_sb, in_=pool_ps)
        pool_ps2 = ppool_sm.tile([P, NR], F32, tag="sm")
        nc.tensor.transpose(pool_ps2, poolT_sb, ident[0:NR, 0:NR])
        pool_sb = segp.tile([P, NSH, 3], F32)
        nc.vector.tensor_copy(out=pool_sb, in_=pool_ps2.rearrange("p (u c) -> p u c", c=3))
        cnt = segp.tile([P, NSH], F32)
        nc.vector.tensor_scalar(out=cnt, in0=pool_sb[:, :, 2], scalar1=1.0, scalar2=None, op0=AL.max)
        rec = segp.tile([P, NSH], F32)
        nc.vector.reciprocal(out=rec, in_=cnt)
        msrcm = segp.tile([P, NSH], F32)
        mtgtm = segp.tile([P, NSH], F32)
        nc.vector.tensor_tensor(out=msrcm, in0=pool_sb[:, :, 0], in1=rec, op=AL.mult)
        nc.vector.tensor_tensor(out=mtgtm, in0=pool_sb[:, :, 1], in1=rec, op=AL.mult)

        msrc_ps = ppool_sm.tile([P, 4], F32, tag="sm")
        nc.tensor.matmul(msrc_ps, lhsT=s1, rhs=msrcm[:, 0:4], start=True, stop=False)
        nc.tensor.matmul(msrc_ps, lhsT=s2, rhs=msrcm[:, 1:5], start=False, stop=True)
        msrc = segp.tile([P, 4], F32)
        nc.vector.tensor_copy(out=msrc, in_=msrc_ps)

        row_ps = ppool_sm.tile([1, 512], F32, tag="sm")
        nc.tensor.matmul(row_ps[:, 0:127], lhsT=mtgtm[:, 4:5], rhs=ident[:, 1:128], start=True, stop=True)
        nc.tensor.matmul(row_ps[:, 127:255], lhsT=mtgtm[:, 5:6], rhs=ident, start=True, stop=True)
        nc.tensor.matmul(row_ps[:, 255:383], lhsT=mtgtm[:, 6:7], rhs=ident, start=True, stop=True)
        nc.tensor.matmul(row_ps[:, 383:511], lhsT=mtgtm[:, 7:8], rhs=ident, start=True, stop=True)
        nc.tensor.matmul(row_ps[:, 511:512], lhsT=mtgtm[:, 8:9], rhs=ident[:, 0:1], start=True, stop=True)
        row_sb = segp.tile([1, 512], F32)
        nc.vector.tensor_scalar(out=row_sb, in0=row_ps, scalar1=bias, scalar2=None, op0=AL.add)

        rowb_ps = ppool_sm.tile([P, 512], F32, tag="sm")
        nc.tensor.matmul(rowb_ps, lhsT=lt[0:1, :], rhs=row_sb, start=True, stop=True)
        rowb = segp.tile([P, 512], F32)
        nc.vector.tensor_copy(out=rowb, in_=rowb_ps)

        for k in range(4):
            lg = opool.tile([P, 512], F32)
            nc.vector.tensor_scalar(out=lg, in0=rowb, scalar1=msrc[:, k:k + 1], scalar2=None, op0=AL.add)
            nc.sync.dma_start(out=y_d[P * k:P * (k + 1), :], in_=lg)

    nc.compile()
    return nc


def _host_prep(inputs):
    x = np.ascontiguousarray(np.asarray(inputs["outputs"], dtype=np.float32))
    wid = np.asarray(inputs["word_ids"]).astype(np.int64)
    cw = np.asarray(inputs["classifier_w"], dtype=np.float32)
    bias = np.float32(np.asarray(inputs["classifier_b"]))
    B, L, Hd = x.shape
    assert (Hd, L) == (H, 4096) and B == 8
    assert int(inputs["num_src"]) == 512 and int(inputs["num_tgt"]) == 512

    # token cutoff: segments beyond 1024 never reach the output
    new_seg = np.ones((B, L), np.int64)
    new_seg[:, 1:] = wid[:, 1:] != wid[:, :-1]
    seg = np.cumsum(new_seg, axis=1) - 1
    cutoff = max(int(np.nonzero(seg[b] <= 1024)[0][-1]) for b in range(B))
    NT = min((cutoff + 1 + P - 1) // P, L // P)
    NT += NT % 2  # even tile count for paired DMA
    NT = min(NT, L // P)
    Ltok = NT * P

    # per-tile projection mode (same compiled program for all cores -> union)
    modes = []
    for i in range(NT):
        smin = int(seg[:, i * P].min())
        smax = int(seg[:, i * P + P - 1].max())
        if smax <= 512:
            modes.append("src")
        elif smin >= 513:
            modes.append("tgt")
        else:
            modes.append("both")

    wrep_b = np.broadcast_to(cw, (P, 2 * H)).astype(np.float32)
    ident = np.eye(P, dtype=np.float32)
    lt = np.triu(np.ones((P, P), np.float32))                   # lt[q,p]=1 iff q<=p
    s1 = np.eye(P, k=-1, dtype=np.float32)                      # s1[q,p]=1 iff q==p+1
    s2 = np.zeros((P, P), np.float32)
    s2[0, P - 1] = 1.0
    iota = np.broadcast_to(np.arange(P, dtype=np.float32), (P, P)).copy()

    in_maps = []
    for b in range(B):
        widf = wid[b, :Ltok].reshape(NT, P).T.astype(np.float32)
        widp = np.concatenate([[-2], wid[b, :Ltok - 1]]).reshape(NT, P).T.astype(np.float32)
        biascol = np.zeros((P, 1), np.float32)
        biascol[0, 0] = bias
        thr = np.broadcast_to(128.0 * np.arange(1, 11, dtype=np.float32), (P, 10))
        cc = np.concatenate([ident, lt, s1, s2, iota, widf, widp, thr, biascol], axis=1)
        in_maps.append({
            "x": np.ascontiguousarray(x[b, :Ltok]),
            "consts": np.ascontiguousarray(cc),
            "wrepb": np.ascontiguousarray(wrep_b),
        })
    return NT, modes, in_maps


def _run(inputs, trace=False, tmpdir=None):
    NT, modes, in_maps = _host_prep(inputs)
    nc = _build_nc(NT, modes)
    res = run_bass_kernel_spmd(nc, in_maps, core_ids=list(range(8)), trace=trace, tmpdir=tmpdir)
    out = np.stack([np.asarray(r["y"], dtype=np.float32) for r in res.results])
    return out, res


def kernel(**inputs) -> np.ndarray:
    out, _ = _run(inputs, trace=False)
    return out


if __name__ == "__main__":
    # CoreSim smoke test on core 0's inputs
    import jax
    jax.config.update("jax_platforms", "cpu")
    sys.path.insert(0, "/root/problem")
    import reference as ref
    from concourse.bass_interp import CoreSim

    inputs = ref.setup_inputs()
    NT, modes, in_maps = _host_prep(inputs)
    print("NT =", NT, "modes:", modes)
    nc = _build_nc(NT, modes)
    sim = CoreSim(nc)
    for name, arr in in_maps[0].items():
        sim.tensor(name)[:] = arr
    sim.simulate()
    got = np.array(sim.tensor("y"))
    expected = np.asarray(ref.reference(**inputs))[0]
    err = np.abs(got - expected).max()
    scale = np.abs(expected).max()
    print("CoreSim abs err:", err, "rel:", err / scale)
    assert err / scale < 1e-2, "CoreSim mismatch"
    print("CORESIM PASSES")
